# revision 10
# baseline (speedup 1.0000x reference)
"""Trainium2 Bass kernel for nn_BiLSTM (3-layer bidirectional LSTM, packed seqs).

Sharding: data-parallel over batch across 8 NeuronCores (8 rows each), both
directions per core, no cross-core communication. Per layer:
  Phase P: input projection Xg = x @ Wih.T as large bf16 matmuls -> DRAM.
  Phase S: sequential scan; per step: scatter-matmul loads Xg into PSUM,
    column-tiled recurrent matmuls accumulate h @ Whh.T, ScalarE activations,
    VectorE cell update, PE transposes produce next step's stationary h.T.
Packed sequences: host-masked inputs + masks folded into tanh scale
(tanh(m*c)) so padded rows emit zeros; final states gathered on host.
"""
import os, sys

for _p in ("/opt/trn_rl_repo", "/root/.axon_site/_ro/trn_rl_repo", "/root/.axon_site"):
    if os.path.isdir(_p) and _p not in sys.path:
        sys.path.append(_p)

import numpy as np
import ml_dtypes

import concourse.bass as bass
import concourse.mybir as mybir
import concourse.tile as tile
from concourse import bacc
from concourse.bass_utils import run_bass_kernel_spmd

dt = mybir.dt
AF = mybir.ActivationFunctionType
BF = dt.bfloat16
F32 = dt.float32
BFNP = ml_dtypes.bfloat16

L, H, D, B, T = 3, 512, 1024, 64, 512
NC = 8
BL = B // NC          # 8 batch rows per core
PROD_Q = T // 16      # 32 production chunks of 16 timesteps

_CACHED = {}


def _build():
    nc = bacc.Bacc(None)
    xtn0_d = nc.declare_dram_parameter("xtn0", [8, 128, T, BL], BF, isOutput=False)
    xtr0_d = nc.declare_dram_parameter("xtr0", [8, 128, T, BL], BF, isOutput=False)
    wih_d = nc.declare_dram_parameter("wih", [L, 128, 2, 8, 2048], BF, isOutput=False)
    whh_d = nc.declare_dram_parameter("whh", [L, 128, 2, 4, 2048], BF, isOutput=False)
    mask_d = nc.declare_dram_parameter("mask", [BL, 2, T], F32, isOutput=False)
    scat_d = nc.declare_dram_parameter("scat", [32, 104], BF, isOutput=False)
    id8_d = nc.declare_dram_parameter("id8", [8, 8], BF, isOutput=False)
    out_d = nc.declare_dram_parameter("out", [T, 2, BL, H], BF, isOutput=True)
    ch_d = nc.declare_dram_parameter("ch", [L, T, 2, BL, H], BF, isOutput=True)
    xtn_d = nc.declare_dram_parameter("xtn", [2, 8, 128, T, BL], BF, isOutput=True)
    xtr_d = nc.dram_tensor("xtr", [2, 8, 128, T, BL], BF)
    xg_d = nc.dram_tensor("xg", [2, T, 2, 4, BL, 512], BF)

    with tile.TileContext(nc) as tc:
        with (
            tc.tile_pool(name="persist", bufs=1) as persist,
            tc.tile_pool(name="wpool", bufs=1) as wpool,
        ):
            mask = persist.tile([40, 2, T], F32, tag="mask")
            nc.default_dma_engine.dma_start(mask[32:40], mask_d[:])
            scat = persist.tile([32, 104], BF, tag="scat")
            nc.default_dma_engine.dma_start(scat, scat_d[:])
            ident = persist.tile([72, 8], BF, tag="ident")
            nc.default_dma_engine.dma_start(ident[64:72], id8_d[:])

            for layer in range(L):
                xg_cur = xg_d[layer % 2]
                # ---------- Phase P: Xg = x @ Wih.T ----------
                wih = wpool.tile([128, 2, 8, 2048], BF, tag="wih")
                nc.default_dma_engine.dma_start(wih, wih_d[layer])
                with (
                    tc.tile_pool(name=f"px{layer}", bufs=3) as pxp,
                    tc.tile_pool(name=f"pe{layer}", bufs=3) as pep,
                    tc.tile_pool(name=f"pps{layer}", bufs=2, space="PSUM") as pps,
                ):
                    for q in range(PROD_Q):
                        for d_ in range(2):
                            xt = pxp.tile([128, 8, 16, BL], BF, tag="xt")
                            if layer == 0:
                                src = (xtn0_d if d_ == 0 else xtr0_d)[:]
                            else:
                                src = (xtn_d if d_ == 0 else xtr_d)[layer % 2]
                            nc.scalar.dma_start(
                                xt, src[:, :, 16 * q:16 * (q + 1), :].rearrange("c p t b -> p c t b"))
                            ps = pps.tile([128, 4, 512], F32, tag="ps")
                            for fk in range(8):
                                for n_ in range(4):
                                    nc.tensor.matmul(
                                        ps[:, n_], xt[:, fk].rearrange("p t b -> p (t b)"),
                                        wih[:, d_, fk, 512 * n_:512 * (n_ + 1)],
                                        start=(fk == 0), stop=(fk == 7),
                                        skip_group_check=True)
                            ev = pep.tile([128, 2048], BF, tag="ev")
                            for n_ in range(4):
                                nc.vector.tensor_copy(ev[:, 512 * n_:512 * (n_ + 1)], ps[:, n_])
                            for n_ in range(4):
                                wbase = xg_cur[16 * q, d_, n_]
                                wdst = bass.AP(
                                    tensor=wbase.tensor, offset=wbase.offset,
                                    ap=[[2 * 4 * BL * 512, 16], [512, BL], [1, 512]])
                                nc.default_dma_engine.dma_start(
                                    wdst, ev[:, 512 * n_:512 * (n_ + 1)])

                # ---------- Phase S: the scan ----------
                whh = wpool.tile([128, 2, 4, 2048], BF, tag="whh")
                nc.default_dma_engine.dma_start(whh, whh_d[layer])
                with (
                    tc.tile_pool(name=f"sx{layer}", bufs=4) as xgp,
                    tc.tile_pool(name=f"ss{layer}", bufs=2) as scratch,
                    tc.tile_pool(name=f"sh{layer}", bufs=3) as hp,
                    tc.tile_pool(name=f"sc{layer}", bufs=3) as cp,
                    tc.tile_pool(name=f"sg{layer}", bufs=2, space="PSUM") as psg,
                    tc.tile_pool(name=f"st{layer}", bufs=2, space="PSUM") as pst,
                ):
                    c_prev = cp.tile([40, 1024], BF, tag="c")
                    nc.vector.memset(c_prev[32:40], 0.0)
                    hT_prev = hp.tile([128, 64], BF, tag="hT")
                    nc.vector.memset(hT_prev, 0.0)

                    for s in range(T):
                        xg = xgp.tile([32, 2, 512], BF, tag="xg")
                        nc.scalar.dma_start(xg, xg_cur[s].rearrange("d j r n -> (j r) d n"))

                        gf = psg.tile([104, 512], F32, tag="gf")
                        gb = psg.tile([104, 512], F32, tag="gb")
                        for d_, g in ((0, gf), (1, gb)):
                            nc.tensor.matmul(g[0:104, :], scat, xg[:, d_],
                                             start=True, stop=False, skip_group_check=True)
                        for k in range(4):
                            for d_, g in ((0, gf), (1, gb)):
                                for j in range(4):
                                    nc.tensor.matmul(
                                        g[32 * j:32 * j + 8, :],
                                        hT_prev[:, 32 * d_ + 8 * k:32 * d_ + 8 * k + 8],
                                        whh[:, d_, k, 512 * j:512 * (j + 1)],
                                        start=False, stop=(k == 3), skip_group_check=True,
                                        tile_position=(0, 32 * j))

                        acts_t = scratch.tile([104, 2048], BF, tag="acts")
                        for d_, g in ((0, gf), (1, gb)):
                            nc.scalar.activation(acts_t[0:104, 512 * d_:512 * (d_ + 1)],
                                                 g[0:104, :], AF.Sigmoid)
                            nc.scalar.activation(acts_t[0:8, 1024 + 512 * d_:1536 + 512 * d_],
                                                 g[96:104, :], AF.Tanh)

                        t12 = scratch.tile([40, 2048], BF, tag="t12")
                        tnh = scratch.tile([72, 1024], BF, tag="tnh")
                        c_new = cp.tile([40, 1024], BF, tag="c")
                        h_t = hp.tile([72, 1024], BF, tag="h")
                        for d_ in range(2):
                            o0 = 512 * d_
                            nc.vector.tensor_mul(t12[32:40, o0:o0 + 512],
                                                 acts_t[32:40, o0:o0 + 512],
                                                 c_prev[32:40, o0:o0 + 512])
                            nc.vector.tensor_mul(t12[32:40, 1024 + o0:1536 + o0],
                                                 acts_t[0:8, o0:o0 + 512],
                                                 acts_t[0:8, 1024 + o0:1536 + o0])
                            nc.vector.tensor_add(c_new[32:40, o0:o0 + 512],
                                                 t12[32:40, o0:o0 + 512],
                                                 t12[32:40, 1024 + o0:1536 + o0])
                            nc.scalar.activation(tnh[64:72, o0:o0 + 512],
                                                 c_new[32:40, o0:o0 + 512], AF.Tanh,
                                                 scale=_mask_scale(nc, mask, d_, s))
                        nc.vector.tensor_mul(h_t[64:72, 0:1024],
                                             acts_t[64:72, 0:1024], tnh[64:72, 0:1024])

                        pT = pst.tile([128, 64], BF, tag="pT")
                        for d_ in range(2):
                            for k in range(4):
                                nc.tensor.transpose(
                                    pT[:, 32 * d_ + 8 * k:32 * d_ + 8 * k + 8],
                                    h_t[64:72, 512 * d_ + 128 * k:512 * d_ + 128 * (k + 1)],
                                    ident[64:72, :])
                        hT_new = hp.tile([128, 64], BF, tag="hT")
                        nc.scalar.copy(hT_new, pT)

                        if layer < L - 1:
                            dst = (layer + 1) % 2
                            # dest order must match src (p, k, b): dims [p, k(chunk), b]
                            def _xt_ap(tens_handle, chunk0, slot):
                                bbase = tens_handle[dst, chunk0, 0, slot, 0]
                                return bass.AP(tensor=bbase.tensor, offset=bbase.offset,
                                               ap=[[T * BL, 128], [128 * T * BL, 4], [1, BL]])
                            nc.gpsimd.dma_start(_xt_ap(xtn_d, 0, s), hT_new[:, 0:32])
                            nc.gpsimd.dma_start(_xt_ap(xtn_d, 4, T - 1 - s), hT_new[:, 32:64])
                            nc.gpsimd.dma_start(_xt_ap(xtr_d, 0, T - 1 - s), hT_new[:, 0:32])
                            nc.gpsimd.dma_start(_xt_ap(xtr_d, 4, s), hT_new[:, 32:64])
                        else:
                            obase = out_d[s, 0, 0, 0]
                            odst = bass.AP(tensor=obase.tensor, offset=obase.offset,
                                           ap=[[H, BL], [BL * H, 2], [1, H]])
                            nc.gpsimd.dma_start(odst, h_t[64:72, :])
                        cbase = ch_d[layer, s, 0, 0, 0]
                        cdst = bass.AP(tensor=cbase.tensor, offset=cbase.offset,
                                       ap=[[H, BL], [BL * H, 2], [1, H]])
                        nc.default_dma_engine.dma_start(cdst, c_new[32:40, :])

                        c_prev = c_new
                        hT_prev = hT_new

    nc.finalize()
    return nc


def _mask_scale(nc, mask, d_, s):
    # per-partition scalar AP [8, 1] at rows 32:40
    return mask[32:40, d_, s:s + 1]


def _prep_inputs(x, lengths):
    x = np.asarray(x, np.float32)
    lengths = np.asarray(lengths, np.int64)
    t_idx = np.arange(T)[:, None]
    active = (t_idx < lengths[None, :])
    xm = np.where(active[:, :, None], x, 0.0)

    scat = np.zeros((32, 104), np.float32)
    for j in range(4):
        for r in range(8):
            scat[8 * j + r, 32 * j + r] = 1.0
    scat = scat.astype(BFNP)
    id8 = np.eye(8, dtype=BFNP)

    per_core = []
    s_idx = np.arange(T)
    for c in range(NC):
        sl = slice(BL * c, BL * (c + 1))
        xc = xm[:, sl, :]
        xT = xc.transpose(2, 0, 1)                      # [1024, T, 8]
        xtn0 = np.ascontiguousarray(xT.reshape(8, 128, T, BL)).astype(BFNP)
        xtr0 = np.ascontiguousarray(xT[:, ::-1, :].reshape(8, 128, T, BL)).astype(BFNP)
        lc = lengths[sl]
        m = np.zeros((BL, 2, T), np.float32)
        for b in range(BL):
            m[b, 0] = (s_idx < lc[b])
            m[b, 1] = ((T - 1 - s_idx) < lc[b])
        per_core.append({"xtn0": xtn0, "xtr0": xtr0, "mask": m,
                         "scat": scat, "id8": id8})
    return per_core


def _prep_weights(Wih, Whh):
    def reorder(w):
        wi, wf, wg, wo = np.split(np.asarray(w, np.float32), 4, axis=-2)
        return np.concatenate([wi, wf, wo, wg], axis=-2)
    WihR = reorder(Wih)   # [L, 2, 2048, 1024]
    WhhR = reorder(Whh)   # [L, 2, 2048, 512]
    # device tile layout [128p, dir, chunk, 2048]: p = feat-within-chunk
    wih_t = np.ascontiguousarray(
        WihR.transpose(0, 1, 3, 2).reshape(L, 2, 8, 128, 2048).transpose(0, 3, 1, 2, 4)).astype(BFNP)
    whh_t = np.ascontiguousarray(
        WhhR.transpose(0, 1, 3, 2).reshape(L, 2, 4, 128, 2048).transpose(0, 3, 1, 2, 4)).astype(BFNP)
    return wih_t, whh_t


def kernel(x, lengths, Wih, Whh, bih, bhh):
    lengths_np = np.asarray(lengths, np.int64)
    bias = np.asarray(bih, np.float32) + np.asarray(bhh, np.float32)
    assert np.abs(bias).max() == 0.0, "nonzero LSTM bias unsupported"

    if "nc" not in _CACHED:
        _CACHED["nc"] = _build()
    nc = _CACHED["nc"]

    wih_t, whh_t = _prep_weights(Wih, Whh)
    per_core = _prep_inputs(x, lengths_np)
    in_maps = [{**pc, "wih": wih_t, "whh": whh_t} for pc in per_core]
    results = run_bass_kernel_spmd(nc, in_maps, list(range(NC))).results

    out = np.zeros((T, B, 2 * H), np.float32)
    h_n = np.zeros((2 * L, B, H), np.float32)
    c_n = np.zeros((2 * L, B, H), np.float32)
    for c in range(NC):
        b0 = BL * c
        r = results[c]
        o = r["out"].astype(np.float32)     # [T, 2, BL, H], dir1 in scan (reversed) order
        ch = r["ch"].astype(np.float32)     # [L, T, 2, BL, H]
        xtn = r["xtn"].astype(np.float32)   # [2, 8, 128, T, BL]
        out[:, b0:b0 + BL, 0:H] = o[:, 0]
        out[:, b0:b0 + BL, H:2 * H] = o[::-1, 1]
        for b in range(BL):
            lb = int(lengths_np[b0 + b])
            h_n[2 * (L - 1), b0 + b] = o[lb - 1, 0, b]
            h_n[2 * (L - 1) + 1, b0 + b] = o[T - 1, 1, b]
            c_n[2 * (L - 1), b0 + b] = ch[L - 1, lb - 1, 0, b]
            c_n[2 * (L - 1) + 1, b0 + b] = ch[L - 1, T - 1, 1, b]
            for layer in range(L - 1):
                buf = (layer + 1) % 2
                h_n[2 * layer, b0 + b] = xtn[buf, 0:4, :, lb - 1, b].reshape(H)
                h_n[2 * layer + 1, b0 + b] = xtn[buf, 4:8, :, 0, b].reshape(H)
                c_n[2 * layer, b0 + b] = ch[layer, lb - 1, 0, b]
                c_n[2 * layer + 1, b0 + b] = ch[layer, T - 1, 1, b]
    return out, h_n, c_n
